# revision 16
# baseline (speedup 1.0000x reference)
"""RGCN (EntityClassifyHeteroAPI) Trainium2 kernel: 3-layer hetero message passing.

Strategy (8 NeuronCores, SPMD single program):
  - Shard destination nodes: core c owns dst rows [c*6250, (c+1)*6250).
  - Per layer: gather messages from a replicated bf16 node-feature table via
    gpsimd.dma_gather (int16 idx; table split at row 25000 into two base APs),
    aggregate per (relation, 128-dst-block) with one-hot matmuls accumulating
    in PSUM (feat-major aggT), per-relation GEMM against W quadrants producing
    node-major output, bias+relu.
  - AllGather (bf16) between layers rebuilds the replicated feature table.
  - Host precomputes chunk schedule, gather indices and one-hot tiles; the
    Bass program is identical across cores (chunk counts are cross-core
    maxima), only input data differs.
"""
import os
import sys

import numpy as np

for _p in ("/opt/trn_rl_repo", "/root/.axon_site/_ro/trn_rl_repo"):
    if _p not in sys.path and os.path.isdir(_p):
        sys.path.insert(0, _p)

import ml_dtypes  # noqa: E402
import concourse.bacc as bacc  # noqa: E402
import concourse.bass as bass  # noqa: E402
import concourse.mybir as mybir  # noqa: E402
import concourse.tile as tile  # noqa: E402
from concourse import bass_utils  # noqa: E402

N_NODES = 50000
H = 256
O = 64
R = 8
E_PER_R = 65536
NCORES = 8
NSH = N_NODES // NCORES  # 6250
SPLIT = 25000            # gather-table base split (int16 idx must stay < 32768)
BLK = 128                # dst nodes per aggregation block
NB = (NSH + BLK - 1) // BLK  # 49 blocks (last has 106 nodes)

BF16 = ml_dtypes.bfloat16


def _ceil_div(a, b):
    return -(-a // b)


def _preprocess(src, dst):
    """Build per-core chunk schedules, gather index arrays and one-hot tiles.

    Edge (r, e) belongs to core c = dst // NSH.  Within a core, edges bucket by
    (block b = dstloc//128, half = src>=SPLIT, relation r) for layers 1/2 and
    by (b, half) with relations merged for layer 0.  Every bucket is padded to
    a whole number of 128-edge chunks; chunk counts are maxima across cores so
    the SPMD program is identical.

    Returns (K12, K0, percore) where
      K12[b][half][r] : chunks for layer-1/2 bucket
      K0[b][half]     : chunks for layer-0 bucket
      percore[c] = dict(gidx12, oh12, gidx0, oh0)  (wrapped idx + onehot arrays)
    """
    src_f = np.asarray(src).reshape(-1).astype(np.int64)
    dst_f = np.asarray(dst).reshape(-1).astype(np.int64)
    rel_f = np.repeat(np.arange(R, dtype=np.int64), E_PER_R)

    core = dst_f // NSH
    dloc = dst_f - core * NSH
    b_of = dloc // BLK
    nloc = dloc - b_of * BLK          # dst offset within block (one-hot column)
    half = (src_f >= SPLIT).astype(np.int64)

    # ---- counts ----
    # layer 1/2 key: (c, b, half, r); layer 0 key: (c, b, half)
    key12 = ((core * NB + b_of) * 2 + half) * R + rel_f
    key0 = (core * NB + b_of) * 2 + half
    cnt12 = np.bincount(key12, minlength=NCORES * NB * 2 * R).reshape(NCORES, NB, 2, R)
    cnt0 = np.bincount(key0, minlength=NCORES * NB * 2).reshape(NCORES, NB, 2)

    K12 = _ceil_div(cnt12, 128).max(axis=0)  # [NB, 2, R]
    K0 = _ceil_div(cnt0, 128).max(axis=0)    # [NB, 2]

    # chunk offsets in the flat chunk list (order: b -> half -> r -> k)
    c12_flat = K12.reshape(-1)
    off12 = np.concatenate([[0], np.cumsum(c12_flat)]).astype(np.int64)
    C12 = int(off12[-1])
    c0_flat = K0.reshape(-1)
    off0 = np.concatenate([[0], np.cumsum(c0_flat)]).astype(np.int64)
    C0 = int(off0[-1])

    percore = []
    for c in range(NCORES):
        m = core == c
        sf, rf, bf, nf, hf = src_f[m], rel_f[m], b_of[m], nloc[m], half[m]

        def build(keys, bucket_of_edge, Kflat, offflat, Ctot):
            """keys: lexsort order; bucket_of_edge: flat bucket id per edge."""
            nbuckets = offflat.size - 1
            order = np.lexsort(keys)
            bk = bucket_of_edge[order]
            s_sorted = sf[order]
            n_sorted = nf[order]
            h_sorted = hf[order]
            # position within bucket
            starts = np.concatenate(
                [[0], np.cumsum(np.bincount(bk, minlength=nbuckets))]
            )
            pos = np.arange(bk.size) - starts[bk]
            chunk = offflat[bk] + pos // 128
            e_in = pos % 128
            slot = chunk * 128 + e_in
            assert (pos // 128 < Kflat[bk]).all(), "chunk overflow"
            S = Ctot * 128
            gidx = np.zeros(S, np.int16)
            gidx[slot] = (s_sorted - h_sorted * SPLIT).astype(np.int16)
            oh = np.zeros((128, Ctot, BLK), np.float32)
            oh[e_in, chunk, n_sorted] = 1.0
            # wrap idx into [128, S//16] (16-partition wrap, replicated x8)
            w = gidx.reshape(-1, 16).T  # [16, S/16]
            w = np.tile(w, (8, 1))      # [128, S/16]
            return np.ascontiguousarray(w), oh.astype(BF16)

        bucket12 = (bf * 2 + hf) * R + rf
        gidx12, oh12 = build((rf, hf, bf), bucket12, c12_flat, off12, C12)
        bucket0 = bf * 2 + hf
        gidx0, oh0 = build((hf, bf), bucket0, c0_flat, off0, C0)
        percore.append(dict(gidx12=gidx12, oh12=oh12, gidx0=gidx0, oh0=oh0))

    return K12, K0, off12, off0, C12, C0, percore


def _build_program(K12, K0, off12, off0, C12, C0, b0z, b1z, b2z):
    """Build the SPMD Bass program (same for all cores)."""
    nc = bacc.Bacc(None, target_bir_lowering=False, debug=False)
    f32, bf16, i16 = mybir.dt.float32, mybir.dt.bfloat16, mybir.dt.int16

    emb = nc.dram_tensor("emb", [N_NODES, H], bf16, kind="ExternalInput")
    w1 = nc.dram_tensor("w1", [R, H, H], bf16, kind="ExternalInput")
    w2 = nc.dram_tensor("w2", [R, H, O], bf16, kind="ExternalInput")
    b0r = nc.dram_tensor("b0r", [128, H], f32, kind="ExternalInput")
    b1r = nc.dram_tensor("b1r", [128, H], f32, kind="ExternalInput")
    b2r = nc.dram_tensor("b2r", [128, O], f32, kind="ExternalInput")
    gidx12_d = nc.dram_tensor("gidx12", [128, C12 * 8], i16, kind="ExternalInput")
    gidx0_d = nc.dram_tensor("gidx0", [128, C0 * 8], i16, kind="ExternalInput")
    oh12_d = nc.dram_tensor("oh12", [128, C12, BLK], bf16, kind="ExternalInput")
    oh0_d = nc.dram_tensor("oh0", [128, C0, BLK], bf16, kind="ExternalInput")
    out_d = nc.dram_tensor("out", [NSH, O], f32, kind="ExternalOutput")

    ag0_in = nc.dram_tensor("ag0_in", [NSH, H], bf16)
    h0 = nc.dram_tensor("h0", [N_NODES, H], bf16, addr_space="Shared")
    ag1_in = nc.dram_tensor("ag1_in", [NSH, H], bf16)
    h1 = nc.dram_tensor("h1", [N_NODES, H], bf16, addr_space="Shared")

    with tile.TileContext(nc) as tc:
        with (
            tc.tile_pool(name="const", bufs=1) as constp,
            tc.tile_pool(name="wpool", bufs=1) as wpool,
            tc.tile_pool(name="msg", bufs=3) as msgp,
            tc.tile_pool(name="ohp", bufs=3) as ohp,
            tc.tile_pool(name="agg", bufs=2) as aggp,
            tc.tile_pool(name="hout", bufs=3) as houtp,
            tc.tile_pool(name="psagg", bufs=4, space="PSUM") as psaggp,
            tc.tile_pool(name="psh", bufs=2, space="PSUM") as pshp,
        ):
            # resident: gather indices, biases
            gidx12_sb = constp.tile([128, C12 * 8], i16, tag="g12")
            nc.sync.dma_start(out=gidx12_sb[:], in_=gidx12_d[:])
            gidx0_sb = constp.tile([128, C0 * 8], i16, tag="g0")
            nc.sync.dma_start(out=gidx0_sb[:], in_=gidx0_d[:])
            bias_sb = {}
            for name, t, width, z in (
                ("b0", b0r, H, b0z), ("b1", b1r, H, b1z), ("b2", b2r, O, b2z)
            ):
                if not z:
                    bias_sb[name] = constp.tile([128, width], f32, tag=name)
                    nc.sync.dma_start(out=bias_sb[name][:], in_=t[:])

            def gather_calls(table, gidx_sb, Kb_half, off_base_chunks, msg_tile):
                """Gather one block's chunks (both halves) into msg_tile.

                Kb_half: [n_lo_chunks, n_hi_chunks]; off_base_chunks: chunk id of
                the block's first chunk.  msg layout: [128, nchunks, H].
                """
                n_lo, n_hi = Kb_half
                col0 = off_base_chunks * 8  # 128 slots/chunk = 8 idx columns
                if n_lo:
                    nidx = n_lo * 128
                    nc.gpsimd.dma_gather(
                        msg_tile[:, 0:n_lo, :],
                        table[:],
                        gidx_sb[:, col0:col0 + n_lo * 8],
                        nidx, nidx, H,
                    )
                if n_hi:
                    nidx = n_hi * 128
                    c1 = col0 + n_lo * 8
                    nc.gpsimd.dma_gather(
                        msg_tile[:, n_lo:n_lo + n_hi, :],
                        table[SPLIT:, :],
                        gidx_sb[:, c1:c1 + n_hi * 8],
                        nidx, nidx, H,
                    )

            # ---------------- layer 0 (no weights, relu) ----------------
            for b in range(NB):
                nrows = min(BLK, NSH - b * BLK)
                n_lo, n_hi = int(K0[b, 0]), int(K0[b, 1])
                ncb = n_lo + n_hi
                cbase = int(off0[b * 2])
                msg = msgp.tile([128, ncb, H], mybir.dt.bfloat16, tag="m0")
                gather_calls(emb, gidx0_sb, (n_lo, n_hi), cbase, msg)
                oh_sb = ohp.tile([128, ncb, BLK], mybir.dt.bfloat16, tag="oh0")
                nc.sync.dma_start(
                    out=oh_sb[:], in_=oh0_d[:, cbase:cbase + ncb, :]
                )
                ps = psaggp.tile([128, H], mybir.dt.float32, space="PSUM", tag="ps")
                for k in range(ncb):
                    nc.tensor.matmul(
                        ps[:],
                        lhsT=oh_sb[:, k, :],
                        rhs=msg[:, k, :],
                        start=(k == 0),
                        stop=(k == ncb - 1),
                    )
                hsb = houtp.tile([128, H], mybir.dt.bfloat16, tag="h0sb")
                if "b0" in bias_sb:
                    tmp = houtp.tile([128, H], mybir.dt.float32, tag="h0tmp")
                    nc.vector.tensor_tensor(
                        out=tmp[:], in0=ps[:], in1=bias_sb["b0"][:],
                        op=mybir.AluOpType.add,
                    )
                    nc.scalar.activation(hsb[:], tmp[:], mybir.ActivationFunctionType.Relu)
                else:
                    nc.scalar.activation(hsb[:], ps[:], mybir.ActivationFunctionType.Relu)
                nc.sync.dma_start(
                    out=ag0_in[b * BLK:b * BLK + nrows, :], in_=hsb[:nrows, :]
                )

            nc.gpsimd.collective_compute(
                "AllGather", mybir.AluOpType.bypass,
                ins=[ag0_in[:]], outs=[h0[:]],
                replica_groups=[list(range(NCORES))],
            )

            # ---------------- layers 1 and 2 ----------------
            def mid_layer(table, w_d, wname, hout_w, bias_key, relu, store):
                # resident W tiles [r][fh] = [128, hout_w] bf16
                wt = []
                for r in range(R):
                    row = []
                    for fh in range(2):
                        t = wpool.tile([128, hout_w], mybir.dt.bfloat16,
                                       tag=f"{wname}_{r}_{fh}")
                        nc.sync.dma_start(
                            out=t[:], in_=w_d[r, fh * 128:(fh + 1) * 128, :]
                        )
                        row.append(t)
                    wt.append(row)

                for b in range(NB):
                    nrows = min(BLK, NSH - b * BLK)
                    ks = [[int(K12[b, hf, r]) for r in range(R)] for hf in range(2)]
                    n_lo = sum(ks[0])
                    n_hi = sum(ks[1])
                    ncb = n_lo + n_hi
                    cbase = int(off12[(b * 2) * R])
                    msg = msgp.tile([128, ncb, H], mybir.dt.bfloat16, tag="m12")
                    gather_calls(table, gidx12_sb, (n_lo, n_hi), cbase, msg)
                    oh_sb = ohp.tile([128, ncb, BLK], mybir.dt.bfloat16, tag="oh12")
                    nc.sync.dma_start(
                        out=oh_sb[:], in_=oh12_d[:, cbase:cbase + ncb, :]
                    )
                    # chunk index (within block) for (hf, r, k)
                    aggs = []
                    for r in range(R):
                        my_chunks = []
                        base_lo = sum(ks[0][:r])
                        for k in range(ks[0][r]):
                            my_chunks.append(base_lo + k)
                        base_hi = n_lo + sum(ks[1][:r])
                        for k in range(ks[1][r]):
                            my_chunks.append(base_hi + k)
                        if not my_chunks:
                            aggs.append(None)
                            continue
                        ps_f = []
                        for _fh in range(2):
                            ps_half = psaggp.tile([128, BLK], mybir.dt.float32,
                                                  space="PSUM", tag="ps")
                            ps_f.append(ps_half)
                        for i, ci in enumerate(my_chunks):
                            for fh in range(2):
                                nc.tensor.matmul(
                                    ps_f[fh][:],
                                    lhsT=msg[:, ci, fh * 128:(fh + 1) * 128],
                                    rhs=oh_sb[:, ci, :],
                                    start=(i == 0),
                                    stop=(i == len(my_chunks) - 1),
                                )
                        agg_sb = aggp.tile([128, H], mybir.dt.bfloat16,
                                           tag=f"agg{r}")
                        for fh in range(2):
                            nc.vector.tensor_copy(
                                out=agg_sb[:, fh * BLK:(fh + 1) * BLK],
                                in_=ps_f[fh][:],
                            )
                        aggs.append(agg_sb)
                    psh = pshp.tile([128, hout_w], mybir.dt.float32, space="PSUM",
                                    tag="psh")
                    live = [(r, fh) for r in range(R) if aggs[r] is not None
                            for fh in range(2)]
                    for i, (r, fh) in enumerate(live):
                        nc.tensor.matmul(
                            psh[:],
                            lhsT=aggs[r][:, fh * BLK:(fh + 1) * BLK],
                            rhs=wt[r][fh][:],
                            start=(i == 0),
                            stop=(i == len(live) - 1),
                        )
                    store(b, nrows, psh, bias_key, relu)

            def store_h(ag_tensor):
                def go(b, nrows, psh, bias_key, relu):
                    hsb = houtp.tile([128, H], mybir.dt.bfloat16, tag="hsb")
                    if bias_key in bias_sb:
                        tmp = houtp.tile([128, H], mybir.dt.float32, tag="htmp")
                        nc.vector.tensor_tensor(
                            out=tmp[:], in0=psh[:], in1=bias_sb[bias_key][:],
                            op=mybir.AluOpType.add,
                        )
                        nc.scalar.activation(hsb[:], tmp[:],
                                             mybir.ActivationFunctionType.Relu)
                    else:
                        nc.scalar.activation(hsb[:], psh[:],
                                             mybir.ActivationFunctionType.Relu)
                    nc.sync.dma_start(
                        out=ag_tensor[b * BLK:b * BLK + nrows, :],
                        in_=hsb[:nrows, :],
                    )
                return go

            def store_out(b, nrows, psh, bias_key, relu):
                osb = houtp.tile([128, O], mybir.dt.float32, tag="osb")
                if bias_key in bias_sb:
                    nc.vector.tensor_tensor(
                        out=osb[:], in0=psh[:], in1=bias_sb[bias_key][:],
                        op=mybir.AluOpType.add,
                    )
                else:
                    nc.vector.tensor_copy(out=osb[:], in_=psh[:])
                nc.sync.dma_start(
                    out=out_d[b * BLK:b * BLK + nrows, :], in_=osb[:nrows, :]
                )

            mid_layer(h0, w1, "w1", H, "b1", True, store_h(ag1_in))
            nc.gpsimd.collective_compute(
                "AllGather", mybir.AluOpType.bypass,
                ins=[ag1_in[:]], outs=[h1[:]],
                replica_groups=[list(range(NCORES))],
            )
            mid_layer(h1, w2, "w2", O, "b2", False, store_out)

    nc.finalize()
    return nc


def _install_ntff_shim():
    """Provide antenv.axon_hooks (missing in this image) so trace=True works."""
    import types
    try:
        from antenv.axon_hooks import get_axon_ntff_profile_hook  # noqa: F401
        return  # real module exists
    except ImportError:
        pass
    mod = types.ModuleType("antenv.axon_hooks")
    state = {"hook": None}
    mod.set_axon_ntff_profile_hook = lambda h: state.__setitem__("hook", h)
    mod.get_axon_ntff_profile_hook = lambda: state["hook"]
    try:
        import antenv
        antenv.axon_hooks = mod
    except ImportError:
        pkg = types.ModuleType("antenv")
        pkg.axon_hooks = mod
        sys.modules["antenv"] = pkg
    sys.modules["antenv.axon_hooks"] = mod
    try:
        from trn_agent_boot.trn_boot import _ntff_profile_via_ctypes
        hook = _ntff_profile_via_ctypes("/opt/axon/libaxon_pjrt.so")
        mod.set_axon_ntff_profile_hook(hook)
    except Exception as e:  # degrade to no tracing
        print(f"[kernel] ntff shim failed: {e}", file=sys.stderr)


def kernel(embed, src, dst, W1, b0, b1, W2, b2):
    embed = np.asarray(embed, dtype=np.float32)
    W1 = np.asarray(W1, dtype=np.float32)
    W2 = np.asarray(W2, dtype=np.float32)
    b0 = np.asarray(b0, dtype=np.float32)
    b1 = np.asarray(b1, dtype=np.float32)
    b2 = np.asarray(b2, dtype=np.float32)

    K12, K0, off12, off0, C12, C0, percore = _preprocess(src, dst)

    nc = _build_program(
        K12, K0, off12, off0, C12, C0,
        bool(np.all(b0 == 0)), bool(np.all(b1 == 0)), bool(np.all(b2 == 0)),
    )

    emb_bf = embed.astype(BF16)
    w1_bf = W1.astype(BF16)
    w2_bf = W2.astype(BF16)
    b0r = np.broadcast_to(b0, (128, H)).copy()
    b1r = np.broadcast_to(b1, (128, H)).copy()
    b2r = np.broadcast_to(b2, (128, O)).copy()

    in_maps = []
    for c in range(NCORES):
        pc = percore[c]
        in_maps.append({
            "emb": emb_bf,
            "w1": w1_bf,
            "w2": w2_bf,
            "b0r": b0r, "b1r": b1r, "b2r": b2r,
            "gidx12": np.ascontiguousarray(pc["gidx12"]),
            "gidx0": np.ascontiguousarray(pc["gidx0"]),
            "oh12": np.ascontiguousarray(pc["oh12"]),
            "oh0": np.ascontiguousarray(pc["oh0"]),
        })

    if int(os.environ.get("BASS_GNN_SIM", "0")):
        import concourse.bass_interp as bass_interp
        sim = bass_interp.MultiCoreSim(nc, NCORES)
        for c in range(NCORES):
            for name, arr in in_maps[c].items():
                sim.cores[c].tensor(name)[:] = arr
        sim.simulate()
        outs = [np.asarray(sim.cores[c].mem_tensor("out")) for c in range(NCORES)]
        return np.concatenate(outs, axis=0).astype(np.float32)

    trace = bool(int(os.environ.get("BASS_GNN_TRACE", "0")))
    if trace:
        _install_ntff_shim()
    res = bass_utils.run_bass_kernel_spmd(
        nc, in_maps, core_ids=list(range(NCORES)), trace=trace,
    )
    if trace and res.exec_time_ns is not None:
        print(f"HW exec time: {res.exec_time_ns} ns")
        kernel.last_exec_time_ns = res.exec_time_ns
    out = np.concatenate([res.results[c]["out"] for c in range(NCORES)], axis=0)
    return out.astype(np.float32)
